# revision 38
# baseline (speedup 1.0000x reference)
"""Chunked DeltaNet layer on 8 TRN2 NeuronCores (v2).

Sharding: core c -> batch b = c//2, head-group hh = c%2 (heads hh*4..hh*4+3).
Each core: q/k/v projections + causal conv + chunked (WY-form) delta rule over
L=2048 in 16 chunks of 128, RMS norm, partial out-projection (contraction over
its 512 local o-dims). Host sums the two partials per batch and adds bo.

v2 redesign vs v1 (601us):
  - All per-token scalars (rk=rsqrt(|k|^2), rq, beta') computed column-major
    [4,512] once per 512-token block, off the per-chunk critical path.
    beta' = beta/(1+beta*|kn|^2) ~= beta/(1+beta) = 0.25 + 0.25*tanh(z/2+ln2/2)
    (eps-drop error ~1e-6; tanh shares the silu activation-table set).
  - k normalized in raw [dim,tok] layout via broadcast-matmul scale tiles, so
    the gram matrix Kn K'^T is built directly and nt's PE transpose gives nm:
    the p3_/pnm-from-scratch/pot PE transposes of v1 are gone.
  - Output computed transposed (po^T = u^T lo + S^T q^T); per-token RMS via
    ones-matmul column sums; rq stays column-major. No output transpose.
  - Head-batched [128,512]/[128,1024] elementwise ops; evacuations split
    across Scalar (copy: table-free in every act set) / DVE / GpSimd.
  - Activation tables: rsqrt (k,q norms early + chunk-phase RMS) and
    silu+tanh (v conv + beta, adjacent) -> 2 table loads per 512-block.

Chunk math per head (state S = [d_k, d_v]):
  G'   = Kn K'^T           Kn = K/|K|, K' = b*Kn, b = beta'
  nt   = triu(G',1) = N^T  nm = N (PE transpose of nt)
  lo   = triu(Kn Q^T, 0)
  x0   = b * [rk*silu(V) | Kn]
  x3   = (I+N^4)(I+N^2)(I-N) x0      (Minv approx, error ~N^8)
  u    = x3_v - x3_k S
  po^T = u^T lo + S^T Q^T
  S   += Kn^T u
  o    = po^T * rq*rsqrt(rq^2*colsum(po^2)/HD + eps)   (RMS + q-norm fold)
"""

import contextlib

import ml_dtypes
import numpy as np

import concourse.bass as bass
import concourse.mybir as mybir
import concourse.tile as tile
from concourse import bacc

F32 = mybir.dt.float32
BF16 = mybir.dt.bfloat16
AF = mybir.ActivationFunctionType
ALU = mybir.AluOpType

B, L, D, H, HD, CONV = 4, 2048, 1024, 8, 128, 4
EPS = 1e-6
C = 128
NCH = L // C
NLT = 4
LT = 512
HL = 4
KS = D // 128
SIG = ("k", "q", "v")
LN2H = 0.34657359  # ln(2)/2


def build_nc():
    nc = bacc.Bacc("TRN2", target_bir_lowering=False, debug=False)

    xt_d = nc.dram_tensor("xt", [KS, 128, L], BF16, kind="ExternalInput").ap()
    wq_d = nc.dram_tensor("wq", [KS, 128, 512], BF16, kind="ExternalInput").ap()
    wk_d = nc.dram_tensor("wk", [KS, 128, 512], BF16, kind="ExternalInput").ap()
    wv_d = nc.dram_tensor("wv", [KS, 128, 512], BF16, kind="ExternalInput").ap()
    wb_d = nc.dram_tensor("wb", [KS, 128, 4], BF16, kind="ExternalInput").ap()
    wo_d = nc.dram_tensor("wo", [4, 128, 1024], BF16, kind="ExternalInput").ap()
    cd_d = nc.dram_tensor("cd", [12, CONV, 128, 128], BF16, kind="ExternalInput").ap()
    mkrep_d = nc.dram_tensor("mkrep", [128, 1024], BF16, kind="ExternalInput").ap()
    oh_d = nc.dram_tensor("oh", [4, 128, 4], BF16, kind="ExternalInput").ap()
    id16_d = nc.dram_tensor("id16", [128, 128], BF16, kind="ExternalInput").ap()
    sel_d = nc.dram_tensor("sel", [16, 16, 128], BF16, kind="ExternalInput").ap()
    out_d = nc.dram_tensor("out", [L, D], F32, kind="ExternalOutput").ap()

    with tile.TileContext(nc) as tc, contextlib.ExitStack() as ctx:
        consts = ctx.enter_context(tc.tile_pool(name="consts", bufs=1))
        persist = ctx.enter_context(tc.tile_pool(name="persist", bufs=1))
        xtp = ctx.enter_context(tc.tile_pool(name="xtp", bufs=2))
        projp = ctx.enter_context(tc.tile_pool(name="projp", bufs=1))
        halop = ctx.enter_context(tc.tile_pool(name="halop", bufs=2))
        rawp = ctx.enter_context(tc.tile_pool(name="rawp", bufs=2))
        bcp = ctx.enter_context(tc.tile_pool(name="bcp", bufs=2))
        sqp = ctx.enter_context(tc.tile_pool(name="sqp", bufs=2))
        scalp = ctx.enter_context(tc.tile_pool(name="scalp", bufs=2))
        chkp = ctx.enter_context(tc.tile_pool(name="chkp", bufs=3))
        chainp = ctx.enter_context(tc.tile_pool(name="chainp", bufs=2))
        outp = ctx.enter_context(tc.tile_pool(name="outp", bufs=2))
        # PSUM banks (8): big 2 + pst 2 + gq 1 + ch_sm 1 + ch_a 2
        ps_big = ctx.enter_context(tc.tile_pool(name="ps_big", bufs=2, space="PSUM"))
        ps_t = ctx.enter_context(tc.tile_pool(name="ps_t", bufs=2, space="PSUM"))
        ps_gq = ctx.enter_context(tc.tile_pool(name="ps_gq", bufs=1, space="PSUM"))
        ps_ch = ctx.enter_context(tc.tile_pool(name="ps_ch", bufs=1, space="PSUM"))
        ps_cha = ctx.enter_context(tc.tile_pool(name="ps_cha", bufs=2, space="PSUM"))

        # ---- constants ----
        ws = {}
        for name, d in (("q", wq_d), ("k", wk_d), ("v", wv_d)):
            w = consts.tile([128, KS, 512], BF16, name=f"w{name}")
            for i in range(KS):
                nc.sync.dma_start(out=w[:, i, :], in_=d[i])
            ws[name] = w
        wb = consts.tile([128, KS, 4], BF16, name="wb")
        for i in range(KS):
            nc.sync.dma_start(out=wb[:, i, :], in_=wb_d[i])
        wo = consts.tile([128, 4, 1024], BF16, name="wo")
        for i in range(4):
            nc.sync.dma_start(out=wo[:, i, :], in_=wo_d[i])
        cd = consts.tile([128, 12, CONV, 128], BF16, name="cd")
        for n_ in range(12):
            for j_ in range(CONV):
                nc.sync.dma_start(out=cd[:, n_, j_, :], in_=cd_d[n_, j_])
        mkrep = consts.tile([128, 8, 128], BF16, name="mkrep")
        for n_ in range(8):
            nc.sync.dma_start(out=mkrep[:, n_, :],
                              in_=mkrep_d[:, n_ * 128:(n_ + 1) * 128])
        oh = consts.tile([128, 4, 4], BF16, name="oh")
        for n_ in range(4):
            nc.sync.dma_start(out=oh[:, n_, :], in_=oh_d[n_])
        id16 = consts.tile([128, 128], BF16, name="id16")
        nc.sync.dma_start(out=id16, in_=id16_d)
        sel = consts.tile([16, 16, 128], BF16, name="sel")
        for n_ in range(16):
            nc.sync.dma_start(out=sel[:, n_, :], in_=sel_d[:, n_])
        onescol = consts.tile([128, 1], BF16, name="onescol")
        nc.vector.memset(onescol, 1.0)
        ln2b = consts.tile([4, 1], F32, name="ln2b")
        nc.vector.memset(ln2b, LN2H)
        epsb = consts.tile([128, 1], F32, name="epsb")
        nc.vector.memset(epsb, EPS)

        # ---- persistent ----
        # per-token column-major scalars, one [4, L] tile each
        rkb_cm = persist.tile([4, L], BF16, name="rkb_cm")
        bp_cm = persist.tile([4, L], BF16, name="bp_cm")
        rq_cm = persist.tile([4, L], BF16, name="rq_cm")
        rk_cm = persist.tile([4, L], BF16, name="rk_cm")
        s16_init = persist.tile([128, HL, 128], BF16, name="s16i")
        nc.vector.memset(s16_init, 0.0)

        prev = {}  # software-pipeline state (conv halos)

        def emit_proj_sig(lt, st, s):
            """projection + conv (+ norm scalars) for one signal s of block lt."""
            tsl = bass.ds(lt * LT, LT)
            si = SIG.index(s)
            if "xt" not in st:
                xt = xtp.tile([128, KS, LT], BF16, name="xt", tag="xt")
                for i in range(KS):
                    nc.sync.dma_start(out=xt[:, i, :], in_=xt_d[i, :, tsl])
                st["xt"] = xt
            xt = st["xt"]
            pts = []
            for h in range(HL):
                ps = ps_big.tile([128, LT], F32, name="psproj", tag="big")
                for i in range(KS):
                    nc.tensor.matmul(
                        ps, ws[s][:, i, h * 128:(h + 1) * 128], xt[:, i, :],
                        start=(i == 0), stop=(i == KS - 1))
                pt = projp.tile([128, LT + 4], BF16, name="pt", tag=f"pj{s}{h}")
                if lt == 0:
                    nc.vector.memset(pt[:, 0:3], 0.0)
                else:
                    nc.vector.tensor_copy(pt[:, 0:3], prev[("halo", s, h)])
                nc.scalar.copy(pt[:, 3:LT + 3], ps)
                halo = halop.tile([128, 3], BF16, name="halo", tag=f"hl{s}{h}")
                nc.gpsimd.tensor_copy(halo, pt[:, LT:LT + 3])
                prev[("halo", s, h)] = halo
                pts.append(pt)
            psn = None
            raw = st["raw"]
            for h in range(HL):
                n = si * HL + h
                pc = ps_big.tile([128, LT], F32, name="psconv", tag="big")
                for j in range(CONV):
                    nc.tensor.matmul(pc, cd[:, n, j, :], pts[h][:, j:LT + j],
                                     start=(j == 0), stop=(j == CONV - 1))
                if s not in raw:
                    raw[s] = rawp.tile([128, HL, LT], BF16, name="raw",
                                       tag=f"rw{s}")
                r = raw[s][:, h, :]
                if s == "v":
                    nc.scalar.activation(r, pc, AF.Silu)
                else:
                    nc.scalar.copy(r, pc)
                    sq = sqp.tile([128, LT], BF16, name="sq", tag="sqs")
                    nc.vector.tensor_mul(sq, r, r)
                    if psn is None:
                        psn = ps_gq.tile([4, LT], F32, name=f"pn{s}", tag="gq")
                    nc.tensor.matmul(psn, oh[:, h, :], sq,
                                     start=(h == 0), stop=(h == HL - 1))
            if s in ("k", "q"):
                nrm = scalp.tile([4, LT], F32, name="nrm", tag="nrm")
                nc.scalar.activation(nrm, psn, AF.Sqrt)
                rf = scalp.tile([4, LT], F32, name="rf", tag="rf")
                nc.vector.reciprocal_approx_fast(rf, nrm)
                dst = rk_cm if s == "k" else rq_cm
                nc.vector.tensor_copy(dst[:, tsl], rf)

        def emit_proj_tail(lt, st):
            """beta projection, beta', rkb and broadcast tiles for block lt."""
            tsl = bass.ds(lt * LT, LT)
            xt = st["xt"]
            psb = ps_gq.tile([4, LT], F32, name="psbeta", tag="gq")
            for i in range(KS):
                nc.tensor.matmul(psb, wb[:, i, :], xt[:, i, :],
                                 start=(i == 0), stop=(i == KS - 1))
            tnh = scalp.tile([4, LT], BF16, name="tnh", tag="tnh")
            nc.scalar.activation(tnh, psb, AF.Tanh, scale=0.5, bias=ln2b)
            nc.vector.tensor_scalar(bp_cm[:, tsl], tnh, 0.25, 0.25,
                                    op0=ALU.mult, op1=ALU.add)
            nc.vector.tensor_mul(rkb_cm[:, tsl], rk_cm[:, tsl], bp_cm[:, tsl])
            for nm_, src_cm in (("rk", rk_cm), ("rkb", rkb_cm)):
                bc = bcp.tile([128, HL, LT], BF16, name=f"{nm_}bc",
                              tag=f"{nm_}bc")
                for h in range(HL):
                    pb = ps_big.tile([128, LT], F32, name="psbc", tag="big")
                    nc.tensor.matmul(pb, sel[0:4, h, :], src_cm[:, tsl])
                    (nc.vector.tensor_copy if h % 2 == 0 else nc.scalar.copy)(
                        bc[:, h, :], pb)
                st[nm_ + "bc"] = bc

        def emit_chunk_a(cidx, st):
            """phase A for chunk cidx; returns state for phase B."""
            cc = cidx % 4
            csl = bass.ds(cc * C, C)
            raw = st["raw"]

            # per-token rows: bt cols 0-3 rkb, 4-7 b, 8-11 rq
            gsl = bass.ds(cidx * C, C)
            psbt = ps_t.tile([128, 12], BF16, name="psbt", tag="pst")
            nc.tensor.transpose(psbt[:, 0:4], rkb_cm[:, gsl], id16[0:4, 0:4])
            nc.tensor.transpose(psbt[:, 4:8], bp_cm[:, gsl], id16[0:4, 0:4])
            nc.tensor.transpose(psbt[:, 8:12], rq_cm[:, gsl], id16[0:4, 0:4])
            bt = chkp.tile([128, 12], F32, name="bt", tag="bt")
            nc.vector.tensor_copy(bt, psbt)

            # normalized k / k' / rk-scaled silu(v), raw [dim, tok] layout
            kn4 = chkp.tile([128, HL, C], BF16, name="kn4", tag="kn4")
            kb4 = chkp.tile([128, HL, C], BF16, name="kb4", tag="kb4")
            vn4 = chkp.tile([128, HL, C], BF16, name="vn4", tag="vn4")
            rkbc, rkbbc = st["rkbc"], st["rkbbc"]
            nc.gpsimd.tensor_mul(kn4, raw["k"][:, :, csl], rkbc[:, :, csl])
            nc.gpsimd.tensor_mul(kb4, raw["k"][:, :, csl], rkbbc[:, :, csl])
            nc.gpsimd.tensor_mul(vn4, raw["v"][:, :, csl], rkbc[:, :, csl])

            # token-major [tok, dim]: vkt = [rk*silu(v) x4 | kn x4]
            pvk = ps_t.tile([128, 8, C], BF16, name="pvk", tag="pst")
            for h in range(HL):
                nc.tensor.transpose(pvk[:, h, :], vn4[:, h, :], id16)
            for h in range(HL):
                nc.tensor.transpose(pvk[:, 4 + h, :], kn4[:, h, :], id16)
            vkt = chkp.tile([128, 8, C], BF16, name="vkt", tag="vkt")
            nc.vector.tensor_copy(vkt, pvk)

            # gram matrices G'_4, KnQ^T_4 -> masked -> [nt_4 | lo_4]
            ntlo = chkp.tile([128, 8, C], BF16, name="ntlo", tag="ntlo")
            pg = ps_gq.tile([128, HL, C], F32, name="pg", tag="gq")
            for h in range(HL):
                nc.tensor.matmul(pg[:, h, :], kn4[:, h, :], kb4[:, h, :])
            nc.vector.tensor_mul(ntlo[:, 0:4, :], pg, mkrep[:, 0:4, :])
            pq = ps_gq.tile([128, HL, C], F32, name="pq", tag="gq")
            for h in range(HL):
                nc.tensor.matmul(pq[:, h, :], kn4[:, h, :],
                                 raw["q"][:, h, csl])
            nc.vector.tensor_mul(ntlo[:, 4:8, :], pq, mkrep[:, 4:8, :])

            # nm = N via PE transpose of nt
            pnm = ps_t.tile([128, HL, C], BF16, name="pnm", tag="pst")
            for h in range(HL):
                nc.tensor.transpose(pnm[:, h, :], ntlo[:, h, :], id16)
            nm4 = chkp.tile([128, HL, C], BF16, name="nm4", tag="nm4")
            nc.vector.tensor_copy(nm4, pnm)

            # x0 = b * [v' | kn]  (gpsimd, per-head halves)
            x = chainp.tile([128, HL, 256], BF16, name="x", tag="x")
            for h in range(HL):
                nc.vector.tensor_scalar_mul(x[:, h, 0:128], vkt[:, h, :],
                                            bt[:, 4 + h:5 + h])
                nc.vector.tensor_scalar_mul(x[:, h, 128:256], vkt[:, 4 + h, :],
                                            bt[:, 4 + h:5 + h])

            # chain matrices: t1 = (N^2)^T, p1 = N^2, t2 = (N^4)^T
            def mm4_copy(lhs_of, rhs_of, name, evac, pool, tag):
                p = pool.tile([128, HL, C], F32, name=f"ps{name}", tag=tag)
                for h in range(HL):
                    nc.tensor.matmul(p[:, h, :], lhs_of(h), rhs_of(h))
                t = chainp.tile([128, HL, C], BF16, name=name, tag=name)
                evac(t, p)
                return t

            t1 = mm4_copy(lambda h: nm4[:, h, :], lambda h: ntlo[:, h, :],
                          "t1", nc.vector.tensor_copy, ps_ch, "ch_sm")
            p1 = mm4_copy(lambda h: ntlo[:, h, :], lambda h: nm4[:, h, :],
                          "p1", nc.scalar.copy, ps_gq, "gq")
            t2 = mm4_copy(lambda h: p1[:, h, :], lambda h: t1[:, h, :],
                          "t2", nc.vector.tensor_copy, ps_ch, "ch_sm")

            # apply chain: x_next = (I + lev^T) x, accumulated on PE
            # (nt mask is negated host-side so level 1 is I - N)
            for li, lev in enumerate((ntlo, t1, t2)):
                xn = chainp.tile([128, HL, 256], BF16, name="x", tag="x")
                for g in range(2):
                    pa = ps_cha.tile([128, 2, 256], F32, name="psa", tag="ch_a")
                    nc.tensor.matmul(pa, id16, x[:, 2 * g:2 * g + 2, :],
                                     start=True, stop=False)
                    for h in (2 * g, 2 * g + 1):
                        nc.tensor.matmul(pa[:, h - 2 * g, :], lev[:, h, :],
                                         x[:, h, :], start=False,
                                         stop=(h == 2 * g + 1))
                    evac = nc.scalar.copy if g == 0 else nc.vector.tensor_copy
                    evac(xn[:, 2 * g:2 * g + 2, :], pa)
                x = xn

            return dict(cidx=cidx, csl=csl, x=x, ntlo=ntlo, vkt=vkt, raw=raw,
                        bt=bt)

        def emit_chunk_b(st, s16_prev, s16_new):
            cidx, csl, x, ntlo, vkt, raw = (
                st["cidx"], st["csl"], st["x"], st["ntlo"], st["vkt"],
                st["raw"])
            # ukt = (x3_k)^T
            pukt = ps_t.tile([128, HL, C], BF16, name="pukt", tag="pst")
            for h in range(HL):
                nc.tensor.transpose(pukt[:, h, :], x[:, h, 128:256], id16)
            ukt = chainp.tile([128, HL, C], BF16, name="ukt", tag="ukt")
            nc.vector.tensor_copy(ukt, pukt)
            # u = x3_v - Uk' S
            pu = ps_ch.tile([128, HL, C], F32, name="psu", tag="ch_sm")
            for h in range(HL):
                nc.tensor.matmul(pu[:, h, :], ukt[:, h, :], s16_prev[:, h, :])
            u4 = chainp.tile([128, HL, C], BF16, name="u4", tag="u4")
            nc.vector.tensor_sub(u4, x[:, :, 0:128], pu)
            # po^T = u^T lo + S^T q^T
            po = ps_cha.tile([128, HL, C], F32, name="pspo", tag="ch_a")
            for h in range(HL):
                nc.tensor.matmul(po[:, h, :], u4[:, h, :], ntlo[:, 4 + h, :],
                                 start=True, stop=False)
                nc.tensor.matmul(po[:, h, :], s16_prev[:, h, :],
                                 raw["q"][:, h, csl], start=False, stop=True)
            # S_new = id*S + Kn^T u  (bf16 state round-trip)
            pd = ps_ch.tile([128, HL, C], F32, name="psd", tag="ch_sm")
            nc.tensor.matmul(pd, id16, s16_prev, start=True, stop=False)
            for h in range(HL):
                nc.tensor.matmul(pd[:, h, :], vkt[:, 4 + h, :], u4[:, h, :],
                                 start=False, stop=(h == HL - 1))
            nc.scalar.copy(s16_new, pd)
            # RMS + q-norm fold, column-wise
            po_sb = chainp.tile([128, HL, C], BF16, name="po_sb", tag="po_sb")
            nc.scalar.copy(po_sb, po)
            sq4 = chainp.tile([128, HL, C], BF16, name="sq4", tag="sq4")
            nc.gpsimd.tensor_mul(sq4, po_sb, po_sb)
            bt = st["bt"]
            pms4 = ps_t.tile([128, 4], F32, name="pms4", tag="pst")
            for h in range(HL):
                nc.tensor.matmul(pms4[:, h:h + 1], sq4[:, h, :], onescol)
            rq2t = chainp.tile([128, 4], BF16, name="rq2t", tag="rq2t")
            nc.vector.tensor_mul(rq2t, bt[:, 8:12], bt[:, 8:12])
            m2t = chainp.tile([128, 4], F32, name="m2t", tag="m2t")
            nc.vector.tensor_mul(m2t, pms4, rq2t)
            ms2 = chainp.tile([128, 4], F32, name="ms2", tag="ms2")
            nc.scalar.activation(ms2, m2t, AF.Sqrt, scale=1.0 / HD, bias=epsb)
            rot = chainp.tile([128, 4], F32, name="rot", tag="rot")
            nc.vector.reciprocal(rot, ms2)
            ro2t = chainp.tile([128, 4], BF16, name="ro2t", tag="ro2t")
            nc.vector.tensor_mul(ro2t, rot, bt[:, 8:12])
            prt = ps_t.tile([4, 128], BF16, name="prt", tag="pst")
            nc.tensor.transpose(prt, ro2t, id16)
            ror = chainp.tile([4, 128], BF16, name="ror", tag="ror")
            nc.vector.tensor_copy(ror, prt)
            prb = ps_t.tile([128, HL, C], F32, name="psrb", tag="pst")
            for h in range(HL):
                nc.tensor.matmul(prb[:, h, :], sel[0:4, h, :], ror)
            ot = chainp.tile([128, HL, C], BF16, name="ot", tag="ot",
                             bufs=10)
            nc.vector.tensor_mul(ot, po_sb, prb)
            return ot

        def emit_outproj(cidx, ot):
            gsl = bass.ds(cidx * C, C)
            for oc in range(2):
                p = ps_cha.tile([128, 512], F32, name="psop", tag="ch_a")
                for h in range(HL):
                    nc.tensor.matmul(p, ot[:, h, :],
                                     wo[:, h, oc * 512:(oc + 1) * 512],
                                     start=(h == 0), stop=(h == HL - 1))
                stt = outp.tile([128, 512], F32, name="ost", tag="ost")
                nc.scalar.copy(stt, p)
                nc.sync.dma_start(out=out_d[gsl, oc * 512:(oc + 1) * 512],
                                  in_=stt)

        def emit_proj_piece(lt, st, piece):
            if piece < 3:
                emit_proj_sig(lt, st, SIG[piece])
            else:
                emit_proj_tail(lt, st)

        pending = None
        s16_prev = s16_init
        opq = []  # deferred outproj queue: (cidx, ot)
        OPLAG = 4

        def flush_outproj(limit):
            while len(opq) > limit:
                ci, ot_ = opq.pop(0)
                emit_outproj(ci, ot_)

        states = {0: {"raw": {}}}
        for piece in range(4):
            emit_proj_piece(0, states[0], piece)
        for lt in range(NLT):
            if lt + 1 < NLT:
                states[lt + 1] = {"raw": {}}
            for cc in range(4):
                if lt + 1 < NLT:
                    emit_proj_piece(lt + 1, states[lt + 1], cc)
                sta = emit_chunk_a(lt * 4 + cc, states[lt])
                if pending is not None:
                    s16_new = chainp.tile([128, HL, 128], BF16, name="s16",
                                          tag="s16")
                    ot = emit_chunk_b(pending, s16_prev, s16_new)
                    opq.append((pending["cidx"], ot))
                    flush_outproj(OPLAG)
                    s16_prev = s16_new
                pending = sta
        s16_new = chainp.tile([128, HL, 128], BF16, name="s16", tag="s16")
        ot = emit_chunk_b(pending, s16_prev, s16_new)
        opq.append((pending["cidx"], ot))
        flush_outproj(0)

    nc.compile()
    return nc


# ---------------- host side ----------------

def _bf(x):
    return np.ascontiguousarray(np.asarray(x, np.float32)).astype(ml_dtypes.bfloat16)


def host_prep(inputs):
    x = np.asarray(inputs["x"], np.float32)
    rms_vec = np.tile(np.asarray(inputs["rms_w"], np.float32), H)
    wo_eff = np.asarray(inputs["Wo"], np.float32) * rms_vec[None, :]

    m1 = np.triu(np.ones((128, 128), np.float32), 1)
    m0 = np.triu(np.ones((128, 128), np.float32), 0)
    mkrep = np.concatenate([np.tile(-m1, (1, 4)), np.tile(m0, (1, 4))],
                           axis=1).astype(np.float32)
    ident = np.eye(128, dtype=np.float32)
    oneh = np.zeros((4, 128, 4), np.float32)
    for h in range(4):
        oneh[h, :, h] = 1.0
    sel3 = np.zeros((16, 16, 128), np.float32)
    for r in range(16):
        sel3[r, r, :] = 1.0

    for nm in ("bq", "bk", "bv", "bbeta", "bo", "convb_q", "convb_k", "convb_v"):
        assert np.all(np.asarray(inputs[nm]) == 0.0), f"nonzero bias {nm} unsupported"

    in_maps = []
    for c in range(8):
        b, hh = c // 2, c % 2
        rows = slice(hh * 512, (hh + 1) * 512)
        cds = []
        for s in ("k", "q", "v"):
            cw = np.asarray(inputs[f"conv_{s}"], np.float32)[rows]
            for h in range(HL):
                cds.append(np.stack([np.diag(cw[h * 128:(h + 1) * 128, j])
                                     for j in range(CONV)]))
        m = {
            "xt": _bf(x[b].T.reshape(KS, 128, L)),
            "wq": _bf(np.asarray(inputs["Wq"], np.float32)[rows].T.reshape(KS, 128, 512)),
            "wk": _bf(np.asarray(inputs["Wk"], np.float32)[rows].T.reshape(KS, 128, 512)),
            "wv": _bf(np.asarray(inputs["Wv"], np.float32)[rows].T.reshape(KS, 128, 512)),
            "wb": _bf(np.asarray(inputs["Wbeta"], np.float32)[hh * 4:(hh + 1) * 4].T.reshape(KS, 128, 4)),
            "wo": _bf(wo_eff[:, rows].T.reshape(4, 128, 1024)),
            "cd": np.stack(cds).astype(ml_dtypes.bfloat16),
            "mkrep": _bf(mkrep),
            "oh": _bf(oneh),
            "id16": _bf(ident),
            "sel": _bf(sel3),
        }
        in_maps.append(m)
    return in_maps


def host_combine(results, inputs):
    bo = np.asarray(inputs["bo"], np.float32)
    out = np.zeros((B, L, D), np.float32)
    for b in range(B):
        out[b] = results[2 * b]["out"] + results[2 * b + 1]["out"] + bo
    return out


# ---------------- entry point ----------------

_NC_CACHE = []


def kernel(**inputs):
    """Full-input DeltaNet layer distributed over 8 NeuronCores.

    Shards batch (4) x head-group (2) across cores, runs the Bass kernel via
    run_bass_kernel_spmd, and reduces the per-pair partial out-projections on
    the host (the pair all-reduce) before returning [4, 2048, 1024] fp32.
    """
    from concourse.bass_utils import run_bass_kernel_spmd

    if not _NC_CACHE:
        _NC_CACHE.append(build_nc())
    nc = _NC_CACHE[0]
    in_maps = host_prep(inputs)
    br = run_bass_kernel_spmd(nc, in_maps, list(range(8)))
    return host_combine(br.results, inputs)


# revision 39
# speedup vs baseline: 1.0196x; 1.0196x over previous
"""Chunked DeltaNet layer on 8 TRN2 NeuronCores (v2).

Sharding: core c -> batch b = c//2, head-group hh = c%2 (heads hh*4..hh*4+3).
Each core: q/k/v projections + causal conv + chunked (WY-form) delta rule over
L=2048 in 16 chunks of 128, RMS norm, partial out-projection (contraction over
its 512 local o-dims). Host sums the two partials per batch and adds bo.

v2 redesign vs v1 (601us):
  - All per-token scalars (rk=rsqrt(|k|^2), rq, beta') computed column-major
    [4,512] once per 512-token block, off the per-chunk critical path.
    beta' = beta/(1+beta*|kn|^2) ~= beta/(1+beta) = 0.25 + 0.25*tanh(z/2+ln2/2)
    (eps-drop error ~1e-6; tanh shares the silu activation-table set).
  - k normalized in raw [dim,tok] layout via broadcast-matmul scale tiles, so
    the gram matrix Kn K'^T is built directly and nt's PE transpose gives nm:
    the p3_/pnm-from-scratch/pot PE transposes of v1 are gone.
  - Output computed transposed (po^T = u^T lo + S^T q^T); per-token RMS via
    ones-matmul column sums; rq stays column-major. No output transpose.
  - Head-batched [128,512]/[128,1024] elementwise ops; evacuations split
    across Scalar (copy: table-free in every act set) / DVE / GpSimd.
  - Activation tables: rsqrt (k,q norms early + chunk-phase RMS) and
    silu+tanh (v conv + beta, adjacent) -> 2 table loads per 512-block.

Chunk math per head (state S = [d_k, d_v]):
  G'   = Kn K'^T           Kn = K/|K|, K' = b*Kn, b = beta'
  nt   = triu(G',1) = N^T  nm = N (PE transpose of nt)
  lo   = triu(Kn Q^T, 0)
  x0   = b * [rk*silu(V) | Kn]
  x3   = (I+N^4)(I+N^2)(I-N) x0      (Minv approx, error ~N^8)
  u    = x3_v - x3_k S
  po^T = u^T lo + S^T Q^T
  S   += Kn^T u
  o    = po^T * rq*rsqrt(rq^2*colsum(po^2)/HD + eps)   (RMS + q-norm fold)
"""

import contextlib

import ml_dtypes
import numpy as np

import concourse.bass as bass
import concourse.mybir as mybir
import concourse.tile as tile
from concourse import bacc

F32 = mybir.dt.float32
BF16 = mybir.dt.bfloat16
AF = mybir.ActivationFunctionType
ALU = mybir.AluOpType

B, L, D, H, HD, CONV = 4, 2048, 1024, 8, 128, 4
EPS = 1e-6
C = 128
NCH = L // C
NLT = 4
LT = 512
HL = 4
KS = D // 128
SIG = ("k", "q", "v")
LN2H = 0.34657359  # ln(2)/2


def build_nc():
    nc = bacc.Bacc("TRN2", target_bir_lowering=False, debug=False)

    xt_d = nc.dram_tensor("xt", [KS, 128, L], BF16, kind="ExternalInput").ap()
    wq_d = nc.dram_tensor("wq", [KS, 128, 512], BF16, kind="ExternalInput").ap()
    wk_d = nc.dram_tensor("wk", [KS, 128, 512], BF16, kind="ExternalInput").ap()
    wv_d = nc.dram_tensor("wv", [KS, 128, 512], BF16, kind="ExternalInput").ap()
    wb_d = nc.dram_tensor("wb", [KS, 128, 4], BF16, kind="ExternalInput").ap()
    wo_d = nc.dram_tensor("wo", [4, 128, 1024], BF16, kind="ExternalInput").ap()
    cd_d = nc.dram_tensor("cd", [12, CONV, 128, 128], BF16, kind="ExternalInput").ap()
    mkrep_d = nc.dram_tensor("mkrep", [128, 1024], BF16, kind="ExternalInput").ap()
    oh_d = nc.dram_tensor("oh", [4, 128, 4], BF16, kind="ExternalInput").ap()
    id16_d = nc.dram_tensor("id16", [128, 128], BF16, kind="ExternalInput").ap()
    sel_d = nc.dram_tensor("sel", [16, 16, 128], BF16, kind="ExternalInput").ap()
    out_d = nc.dram_tensor("out", [L, D], F32, kind="ExternalOutput").ap()

    with tile.TileContext(nc) as tc, contextlib.ExitStack() as ctx:
        consts = ctx.enter_context(tc.tile_pool(name="consts", bufs=1))
        persist = ctx.enter_context(tc.tile_pool(name="persist", bufs=1))
        xtp = ctx.enter_context(tc.tile_pool(name="xtp", bufs=2))
        projp = ctx.enter_context(tc.tile_pool(name="projp", bufs=1))
        halop = ctx.enter_context(tc.tile_pool(name="halop", bufs=2))
        rawp = ctx.enter_context(tc.tile_pool(name="rawp", bufs=2))
        bcp = ctx.enter_context(tc.tile_pool(name="bcp", bufs=2))
        sqp = ctx.enter_context(tc.tile_pool(name="sqp", bufs=2))
        scalp = ctx.enter_context(tc.tile_pool(name="scalp", bufs=2))
        chkp = ctx.enter_context(tc.tile_pool(name="chkp", bufs=3))
        chainp = ctx.enter_context(tc.tile_pool(name="chainp", bufs=2))
        outp = ctx.enter_context(tc.tile_pool(name="outp", bufs=2))
        # PSUM banks (8): big 2 + pst 2 + gq 1 + ch_sm 1 + ch_a 2
        ps_big = ctx.enter_context(tc.tile_pool(name="ps_big", bufs=2, space="PSUM"))
        ps_t = ctx.enter_context(tc.tile_pool(name="ps_t", bufs=2, space="PSUM"))
        ps_gq = ctx.enter_context(tc.tile_pool(name="ps_gq", bufs=1, space="PSUM"))
        ps_ch = ctx.enter_context(tc.tile_pool(name="ps_ch", bufs=1, space="PSUM"))
        ps_cha = ctx.enter_context(tc.tile_pool(name="ps_cha", bufs=2, space="PSUM"))

        # ---- constants ----
        ws = {}
        for name, d in (("q", wq_d), ("k", wk_d), ("v", wv_d)):
            w = consts.tile([128, KS, 512], BF16, name=f"w{name}")
            for i in range(KS):
                nc.sync.dma_start(out=w[:, i, :], in_=d[i])
            ws[name] = w
        wb = consts.tile([128, KS, 4], BF16, name="wb")
        for i in range(KS):
            nc.sync.dma_start(out=wb[:, i, :], in_=wb_d[i])
        wo = consts.tile([128, 4, 1024], BF16, name="wo")
        for i in range(4):
            nc.sync.dma_start(out=wo[:, i, :], in_=wo_d[i])
        cd = consts.tile([128, 12, CONV, 128], BF16, name="cd")
        for n_ in range(12):
            for j_ in range(CONV):
                nc.sync.dma_start(out=cd[:, n_, j_, :], in_=cd_d[n_, j_])
        mkrep = consts.tile([128, 8, 128], BF16, name="mkrep")
        for n_ in range(8):
            nc.sync.dma_start(out=mkrep[:, n_, :],
                              in_=mkrep_d[:, n_ * 128:(n_ + 1) * 128])
        oh = consts.tile([128, 4, 4], BF16, name="oh")
        for n_ in range(4):
            nc.sync.dma_start(out=oh[:, n_, :], in_=oh_d[n_])
        id16 = consts.tile([128, 128], BF16, name="id16")
        nc.sync.dma_start(out=id16, in_=id16_d)
        sel = consts.tile([16, 16, 128], BF16, name="sel")
        for n_ in range(16):
            nc.sync.dma_start(out=sel[:, n_, :], in_=sel_d[:, n_])
        onescol = consts.tile([128, 1], BF16, name="onescol")
        nc.vector.memset(onescol, 1.0)
        ln2b = consts.tile([4, 1], F32, name="ln2b")
        nc.vector.memset(ln2b, LN2H)
        epsb = consts.tile([128, 1], F32, name="epsb")
        nc.vector.memset(epsb, EPS)

        # ---- persistent ----
        # per-token column-major scalars, one [4, L] tile each
        rkb_cm = persist.tile([4, L], BF16, name="rkb_cm")
        bp_cm = persist.tile([4, L], BF16, name="bp_cm")
        rq_cm = persist.tile([4, L], BF16, name="rq_cm")
        rk_cm = persist.tile([4, L], BF16, name="rk_cm")
        s16_init = persist.tile([128, HL, 128], BF16, name="s16i")
        nc.vector.memset(s16_init, 0.0)

        prev = {}  # software-pipeline state (conv halos)

        def emit_proj_sig(lt, st, s):
            """projection + conv (+ norm scalars) for one signal s of block lt."""
            tsl = bass.ds(lt * LT, LT)
            si = SIG.index(s)
            if "xt" not in st:
                xt = xtp.tile([128, KS, LT], BF16, name="xt", tag="xt")
                for i in range(KS):
                    nc.sync.dma_start(out=xt[:, i, :], in_=xt_d[i, :, tsl])
                st["xt"] = xt
            xt = st["xt"]
            pts = []
            for h in range(HL):
                ps = ps_big.tile([128, LT], F32, name="psproj", tag="big")
                for i in range(KS):
                    nc.tensor.matmul(
                        ps, ws[s][:, i, h * 128:(h + 1) * 128], xt[:, i, :],
                        start=(i == 0), stop=(i == KS - 1))
                pt = projp.tile([128, LT + 4], BF16, name="pt", tag=f"pj{s}{h}")
                if lt == 0:
                    nc.vector.memset(pt[:, 0:3], 0.0)
                else:
                    nc.vector.tensor_copy(pt[:, 0:3], prev[("halo", s, h)])
                nc.scalar.copy(pt[:, 3:LT + 3], ps)
                halo = halop.tile([128, 3], BF16, name="halo", tag=f"hl{s}{h}")
                nc.gpsimd.tensor_copy(halo, pt[:, LT:LT + 3])
                prev[("halo", s, h)] = halo
                pts.append(pt)
            psn = None
            raw = st["raw"]
            for h in range(HL):
                n = si * HL + h
                pc = ps_big.tile([128, LT], F32, name="psconv", tag="big")
                for j in range(CONV):
                    nc.tensor.matmul(pc, cd[:, n, j, :], pts[h][:, j:LT + j],
                                     start=(j == 0), stop=(j == CONV - 1))
                if s not in raw:
                    raw[s] = rawp.tile([128, HL, LT], BF16, name="raw",
                                       tag=f"rw{s}")
                r = raw[s][:, h, :]
                if s == "v":
                    nc.scalar.activation(r, pc, AF.Silu)
                else:
                    nc.scalar.copy(r, pc)
                    sq = sqp.tile([128, LT], BF16, name="sq", tag="sqs")
                    nc.vector.tensor_mul(sq, r, r)
                    if psn is None:
                        psn = ps_gq.tile([4, LT], F32, name=f"pn{s}", tag="gq")
                    nc.tensor.matmul(psn, oh[:, h, :], sq,
                                     start=(h == 0), stop=(h == HL - 1))
            if s in ("k", "q"):
                nrm = scalp.tile([4, LT], F32, name="nrm", tag="nrm")
                nc.scalar.activation(nrm, psn, AF.Sqrt)
                rf = scalp.tile([4, LT], F32, name="rf", tag="rf")
                nc.vector.reciprocal_approx_fast(rf, nrm)
                dst = rk_cm if s == "k" else rq_cm
                nc.vector.tensor_copy(dst[:, tsl], rf)

        def emit_proj_tail(lt, st):
            """beta projection, beta', rkb and broadcast tiles for block lt."""
            tsl = bass.ds(lt * LT, LT)
            xt = st["xt"]
            psb = ps_gq.tile([4, LT], F32, name="psbeta", tag="gq")
            for i in range(KS):
                nc.tensor.matmul(psb, wb[:, i, :], xt[:, i, :],
                                 start=(i == 0), stop=(i == KS - 1))
            tnh = scalp.tile([4, LT], BF16, name="tnh", tag="tnh")
            nc.scalar.activation(tnh, psb, AF.Tanh, scale=0.5, bias=ln2b)
            nc.vector.tensor_scalar(bp_cm[:, tsl], tnh, 0.25, 0.25,
                                    op0=ALU.mult, op1=ALU.add)
            nc.vector.tensor_mul(rkb_cm[:, tsl], rk_cm[:, tsl], bp_cm[:, tsl])
            for nm_, src_cm in (("rk", rk_cm), ("rkb", rkb_cm)):
                bc = bcp.tile([128, HL, LT], BF16, name=f"{nm_}bc",
                              tag=f"{nm_}bc")
                for h in range(HL):
                    pb = ps_big.tile([128, LT], F32, name="psbc", tag="big")
                    nc.tensor.matmul(pb, sel[0:4, h, :], src_cm[:, tsl])
                    (nc.vector.tensor_copy if h % 2 == 0 else nc.scalar.copy)(
                        bc[:, h, :], pb)
                st[nm_ + "bc"] = bc

        def emit_chunk_a(cidx, st):
            """phase A for chunk cidx; returns state for phase B."""
            cc = cidx % 4
            csl = bass.ds(cc * C, C)
            raw = st["raw"]

            # per-token rows: bt cols 0-3 rkb, 4-7 b, 8-11 rq
            gsl = bass.ds(cidx * C, C)
            psbt = ps_t.tile([128, 12], BF16, name="psbt", tag="pst")
            nc.tensor.transpose(psbt[:, 0:4], rkb_cm[:, gsl], id16[0:4, 0:4])
            nc.tensor.transpose(psbt[:, 4:8], bp_cm[:, gsl], id16[0:4, 0:4])
            nc.tensor.transpose(psbt[:, 8:12], rq_cm[:, gsl], id16[0:4, 0:4])
            bt = chkp.tile([128, 12], F32, name="bt", tag="bt")
            nc.vector.tensor_copy(bt, psbt)

            # normalized k / k' / rk-scaled silu(v), raw [dim, tok] layout
            kn4 = chkp.tile([128, HL, C], BF16, name="kn4", tag="kn4")
            kb4 = chkp.tile([128, HL, C], BF16, name="kb4", tag="kb4")
            vn4 = chkp.tile([128, HL, C], BF16, name="vn4", tag="vn4")
            rkbc, rkbbc = st["rkbc"], st["rkbbc"]
            nc.vector.tensor_mul(kn4, raw["k"][:, :, csl], rkbc[:, :, csl])
            nc.vector.tensor_mul(kb4, raw["k"][:, :, csl], rkbbc[:, :, csl])
            nc.gpsimd.tensor_mul(vn4, raw["v"][:, :, csl], rkbc[:, :, csl])

            # token-major [tok, dim]: vkt = [rk*silu(v) x4 | kn x4]
            pvk = ps_t.tile([128, 8, C], BF16, name="pvk", tag="pst")
            for h in range(HL):
                nc.tensor.transpose(pvk[:, h, :], vn4[:, h, :], id16)
            for h in range(HL):
                nc.tensor.transpose(pvk[:, 4 + h, :], kn4[:, h, :], id16)
            vkt = chkp.tile([128, 8, C], BF16, name="vkt", tag="vkt")
            nc.vector.tensor_copy(vkt, pvk)

            # gram matrices G'_4, KnQ^T_4 -> masked -> [nt_4 | lo_4]
            ntlo = chkp.tile([128, 8, C], BF16, name="ntlo", tag="ntlo")
            pg = ps_gq.tile([128, HL, C], F32, name="pg", tag="gq")
            for h in range(HL):
                nc.tensor.matmul(pg[:, h, :], kn4[:, h, :], kb4[:, h, :])
            nc.vector.tensor_mul(ntlo[:, 0:4, :], pg, mkrep[:, 0:4, :])
            pq = ps_gq.tile([128, HL, C], F32, name="pq", tag="gq")
            for h in range(HL):
                nc.tensor.matmul(pq[:, h, :], kn4[:, h, :],
                                 raw["q"][:, h, csl])
            nc.vector.tensor_mul(ntlo[:, 4:8, :], pq, mkrep[:, 4:8, :])

            # nm = N via PE transpose of nt
            pnm = ps_t.tile([128, HL, C], BF16, name="pnm", tag="pst")
            for h in range(HL):
                nc.tensor.transpose(pnm[:, h, :], ntlo[:, h, :], id16)
            nm4 = chkp.tile([128, HL, C], BF16, name="nm4", tag="nm4")
            nc.vector.tensor_copy(nm4, pnm)

            # x0 = b * [v' | kn]  (gpsimd, per-head halves)
            x = chainp.tile([128, HL, 256], BF16, name="x", tag="x")
            for h in range(HL):
                nc.vector.tensor_scalar_mul(x[:, h, 0:128], vkt[:, h, :],
                                            bt[:, 4 + h:5 + h])
                nc.vector.tensor_scalar_mul(x[:, h, 128:256], vkt[:, 4 + h, :],
                                            bt[:, 4 + h:5 + h])

            # chain matrices: t1 = (N^2)^T, p1 = N^2, t2 = (N^4)^T
            def mm4_copy(lhs_of, rhs_of, name, evac, pool, tag):
                p = pool.tile([128, HL, C], F32, name=f"ps{name}", tag=tag)
                for h in range(HL):
                    nc.tensor.matmul(p[:, h, :], lhs_of(h), rhs_of(h))
                t = chainp.tile([128, HL, C], BF16, name=name, tag=name)
                evac(t, p)
                return t

            t1 = mm4_copy(lambda h: nm4[:, h, :], lambda h: ntlo[:, h, :],
                          "t1", nc.vector.tensor_copy, ps_ch, "ch_sm")
            p1 = mm4_copy(lambda h: ntlo[:, h, :], lambda h: nm4[:, h, :],
                          "p1", nc.scalar.copy, ps_gq, "gq")
            t2 = mm4_copy(lambda h: p1[:, h, :], lambda h: t1[:, h, :],
                          "t2", nc.vector.tensor_copy, ps_ch, "ch_sm")

            # apply chain: x_next = (I + lev^T) x, accumulated on PE
            # (nt mask is negated host-side so level 1 is I - N)
            for li, lev in enumerate((ntlo, t1, t2)):
                xn = chainp.tile([128, HL, 256], BF16, name="x", tag="x")
                for g in range(2):
                    pa = ps_cha.tile([128, 2, 256], F32, name="psa", tag="ch_a")
                    nc.tensor.matmul(pa, id16, x[:, 2 * g:2 * g + 2, :],
                                     start=True, stop=False)
                    for h in (2 * g, 2 * g + 1):
                        nc.tensor.matmul(pa[:, h - 2 * g, :], lev[:, h, :],
                                         x[:, h, :], start=False,
                                         stop=(h == 2 * g + 1))
                    evac = nc.scalar.copy if g == 0 else nc.vector.tensor_copy
                    evac(xn[:, 2 * g:2 * g + 2, :], pa)
                x = xn

            return dict(cidx=cidx, csl=csl, x=x, ntlo=ntlo, vkt=vkt, raw=raw,
                        bt=bt)

        def emit_chunk_b(st, s16_prev, s16_new):
            cidx, csl, x, ntlo, vkt, raw = (
                st["cidx"], st["csl"], st["x"], st["ntlo"], st["vkt"],
                st["raw"])
            # ukt = (x3_k)^T
            pukt = ps_t.tile([128, HL, C], BF16, name="pukt", tag="pst")
            for h in range(HL):
                nc.tensor.transpose(pukt[:, h, :], x[:, h, 128:256], id16)
            ukt = chainp.tile([128, HL, C], BF16, name="ukt", tag="ukt")
            nc.vector.tensor_copy(ukt, pukt)
            # u = x3_v - Uk' S
            pu = ps_ch.tile([128, HL, C], F32, name="psu", tag="ch_sm")
            for h in range(HL):
                nc.tensor.matmul(pu[:, h, :], ukt[:, h, :], s16_prev[:, h, :])
            u4 = chainp.tile([128, HL, C], BF16, name="u4", tag="u4")
            nc.vector.tensor_sub(u4, x[:, :, 0:128], pu)
            # po^T = u^T lo + S^T q^T
            po = ps_cha.tile([128, HL, C], F32, name="pspo", tag="ch_a")
            for h in range(HL):
                nc.tensor.matmul(po[:, h, :], u4[:, h, :], ntlo[:, 4 + h, :],
                                 start=True, stop=False)
                nc.tensor.matmul(po[:, h, :], s16_prev[:, h, :],
                                 raw["q"][:, h, csl], start=False, stop=True)
            # S_new = id*S + Kn^T u  (bf16 state round-trip)
            pd = ps_ch.tile([128, HL, C], F32, name="psd", tag="ch_sm")
            nc.tensor.matmul(pd, id16, s16_prev, start=True, stop=False)
            for h in range(HL):
                nc.tensor.matmul(pd[:, h, :], vkt[:, 4 + h, :], u4[:, h, :],
                                 start=False, stop=(h == HL - 1))
            nc.scalar.copy(s16_new, pd)
            # RMS + q-norm fold, column-wise
            po_sb = chainp.tile([128, HL, C], BF16, name="po_sb", tag="po_sb")
            nc.scalar.copy(po_sb, po)
            sq4 = chainp.tile([128, HL, C], BF16, name="sq4", tag="sq4")
            nc.gpsimd.tensor_mul(sq4, po_sb, po_sb)
            bt = st["bt"]
            pms4 = ps_t.tile([128, 4], F32, name="pms4", tag="pst")
            for h in range(HL):
                nc.tensor.matmul(pms4[:, h:h + 1], sq4[:, h, :], onescol)
            rq2t = chainp.tile([128, 4], BF16, name="rq2t", tag="rq2t")
            nc.vector.tensor_mul(rq2t, bt[:, 8:12], bt[:, 8:12])
            m2t = chainp.tile([128, 4], F32, name="m2t", tag="m2t")
            nc.vector.tensor_mul(m2t, pms4, rq2t)
            ms2 = chainp.tile([128, 4], F32, name="ms2", tag="ms2")
            nc.scalar.activation(ms2, m2t, AF.Sqrt, scale=1.0 / HD, bias=epsb)
            rot = chainp.tile([128, 4], F32, name="rot", tag="rot")
            nc.vector.reciprocal(rot, ms2)
            ro2t = chainp.tile([128, 4], BF16, name="ro2t", tag="ro2t")
            nc.vector.tensor_mul(ro2t, rot, bt[:, 8:12])
            prt = ps_t.tile([4, 128], BF16, name="prt", tag="pst")
            nc.tensor.transpose(prt, ro2t, id16)
            ror = chainp.tile([4, 128], BF16, name="ror", tag="ror")
            nc.vector.tensor_copy(ror, prt)
            prb = ps_t.tile([128, HL, C], F32, name="psrb", tag="pst")
            for h in range(HL):
                nc.tensor.matmul(prb[:, h, :], sel[0:4, h, :], ror)
            ot = chainp.tile([128, HL, C], BF16, name="ot", tag="ot",
                             bufs=10)
            nc.vector.tensor_mul(ot, po_sb, prb)
            return ot

        def emit_outproj(cidx, ot):
            gsl = bass.ds(cidx * C, C)
            for oc in range(2):
                p = ps_cha.tile([128, 512], F32, name="psop", tag="ch_a")
                for h in range(HL):
                    nc.tensor.matmul(p, ot[:, h, :],
                                     wo[:, h, oc * 512:(oc + 1) * 512],
                                     start=(h == 0), stop=(h == HL - 1))
                stt = outp.tile([128, 512], F32, name="ost", tag="ost")
                nc.scalar.copy(stt, p)
                nc.sync.dma_start(out=out_d[gsl, oc * 512:(oc + 1) * 512],
                                  in_=stt)

        def emit_proj_piece(lt, st, piece):
            if piece < 3:
                emit_proj_sig(lt, st, SIG[piece])
            else:
                emit_proj_tail(lt, st)

        pending = None
        s16_prev = s16_init
        opq = []  # deferred outproj queue: (cidx, ot)
        OPLAG = 4

        def flush_outproj(limit):
            while len(opq) > limit:
                ci, ot_ = opq.pop(0)
                emit_outproj(ci, ot_)

        states = {0: {"raw": {}}}
        for piece in range(4):
            emit_proj_piece(0, states[0], piece)
        for lt in range(NLT):
            if lt + 1 < NLT:
                states[lt + 1] = {"raw": {}}
            for cc in range(4):
                if lt + 1 < NLT:
                    emit_proj_piece(lt + 1, states[lt + 1], cc)
                sta = emit_chunk_a(lt * 4 + cc, states[lt])
                if pending is not None:
                    s16_new = chainp.tile([128, HL, 128], BF16, name="s16",
                                          tag="s16")
                    ot = emit_chunk_b(pending, s16_prev, s16_new)
                    opq.append((pending["cidx"], ot))
                    flush_outproj(OPLAG)
                    s16_prev = s16_new
                pending = sta
        s16_new = chainp.tile([128, HL, 128], BF16, name="s16", tag="s16")
        ot = emit_chunk_b(pending, s16_prev, s16_new)
        opq.append((pending["cidx"], ot))
        flush_outproj(0)

    nc.compile()
    return nc


# ---------------- host side ----------------

def _bf(x):
    return np.ascontiguousarray(np.asarray(x, np.float32)).astype(ml_dtypes.bfloat16)


def host_prep(inputs):
    x = np.asarray(inputs["x"], np.float32)
    rms_vec = np.tile(np.asarray(inputs["rms_w"], np.float32), H)
    wo_eff = np.asarray(inputs["Wo"], np.float32) * rms_vec[None, :]

    m1 = np.triu(np.ones((128, 128), np.float32), 1)
    m0 = np.triu(np.ones((128, 128), np.float32), 0)
    mkrep = np.concatenate([np.tile(-m1, (1, 4)), np.tile(m0, (1, 4))],
                           axis=1).astype(np.float32)
    ident = np.eye(128, dtype=np.float32)
    oneh = np.zeros((4, 128, 4), np.float32)
    for h in range(4):
        oneh[h, :, h] = 1.0
    sel3 = np.zeros((16, 16, 128), np.float32)
    for r in range(16):
        sel3[r, r, :] = 1.0

    for nm in ("bq", "bk", "bv", "bbeta", "bo", "convb_q", "convb_k", "convb_v"):
        assert np.all(np.asarray(inputs[nm]) == 0.0), f"nonzero bias {nm} unsupported"

    in_maps = []
    for c in range(8):
        b, hh = c // 2, c % 2
        rows = slice(hh * 512, (hh + 1) * 512)
        cds = []
        for s in ("k", "q", "v"):
            cw = np.asarray(inputs[f"conv_{s}"], np.float32)[rows]
            for h in range(HL):
                cds.append(np.stack([np.diag(cw[h * 128:(h + 1) * 128, j])
                                     for j in range(CONV)]))
        m = {
            "xt": _bf(x[b].T.reshape(KS, 128, L)),
            "wq": _bf(np.asarray(inputs["Wq"], np.float32)[rows].T.reshape(KS, 128, 512)),
            "wk": _bf(np.asarray(inputs["Wk"], np.float32)[rows].T.reshape(KS, 128, 512)),
            "wv": _bf(np.asarray(inputs["Wv"], np.float32)[rows].T.reshape(KS, 128, 512)),
            "wb": _bf(np.asarray(inputs["Wbeta"], np.float32)[hh * 4:(hh + 1) * 4].T.reshape(KS, 128, 4)),
            "wo": _bf(wo_eff[:, rows].T.reshape(4, 128, 1024)),
            "cd": np.stack(cds).astype(ml_dtypes.bfloat16),
            "mkrep": _bf(mkrep),
            "oh": _bf(oneh),
            "id16": _bf(ident),
            "sel": _bf(sel3),
        }
        in_maps.append(m)
    return in_maps


def host_combine(results, inputs):
    bo = np.asarray(inputs["bo"], np.float32)
    out = np.zeros((B, L, D), np.float32)
    for b in range(B):
        out[b] = results[2 * b]["out"] + results[2 * b + 1]["out"] + bo
    return out


# ---------------- entry point ----------------

_NC_CACHE = []


def kernel(**inputs):
    """Full-input DeltaNet layer distributed over 8 NeuronCores.

    Shards batch (4) x head-group (2) across cores, runs the Bass kernel via
    run_bass_kernel_spmd, and reduces the per-pair partial out-projections on
    the host (the pair all-reduce) before returning [4, 2048, 1024] fp32.
    """
    from concourse.bass_utils import run_bass_kernel_spmd

    if not _NC_CACHE:
        _NC_CACHE.append(build_nc())
    nc = _NC_CACHE[0]
    in_maps = host_prep(inputs)
    br = run_bass_kernel_spmd(nc, in_maps, list(range(8)))
    return host_combine(br.results, inputs)


# revision 45
# speedup vs baseline: 1.0371x; 1.0171x over previous
"""Chunked DeltaNet layer on 8 TRN2 NeuronCores (v2).

Sharding: core c -> batch b = c//2, head-group hh = c%2 (heads hh*4..hh*4+3).
Each core: q/k/v projections + causal conv + chunked (WY-form) delta rule over
L=2048 in 16 chunks of 128, RMS norm, partial out-projection (contraction over
its 512 local o-dims). Host sums the two partials per batch and adds bo.

v2 redesign vs v1 (601us):
  - All per-token scalars (rk=rsqrt(|k|^2), rq, beta') computed column-major
    [4,512] once per 512-token block, off the per-chunk critical path.
    beta' = beta/(1+beta*|kn|^2) ~= beta/(1+beta) = 0.25 + 0.25*tanh(z/2+ln2/2)
    (eps-drop error ~1e-6; tanh shares the silu activation-table set).
  - k normalized in raw [dim,tok] layout via broadcast-matmul scale tiles, so
    the gram matrix Kn K'^T is built directly and nt's PE transpose gives nm:
    the p3_/pnm-from-scratch/pot PE transposes of v1 are gone.
  - Output computed transposed (po^T = u^T lo + S^T q^T); per-token RMS via
    ones-matmul column sums; rq stays column-major. No output transpose.
  - Head-batched [128,512]/[128,1024] elementwise ops; evacuations split
    across Scalar (copy: table-free in every act set) / DVE / GpSimd.
  - Activation tables: rsqrt (k,q norms early + chunk-phase RMS) and
    silu+tanh (v conv + beta, adjacent) -> 2 table loads per 512-block.

Chunk math per head (state S = [d_k, d_v]):
  G'   = Kn K'^T           Kn = K/|K|, K' = b*Kn, b = beta'
  nt   = triu(G',1) = N^T  nm = N (PE transpose of nt)
  lo   = triu(Kn Q^T, 0)
  x0   = b * [rk*silu(V) | Kn]
  x3   = (I+N^4)(I+N^2)(I-N) x0      (Minv approx, error ~N^8)
  u    = x3_v - x3_k S
  po^T = u^T lo + S^T Q^T
  S   += Kn^T u
  o    = po^T * rq*rsqrt(rq^2*colsum(po^2)/HD + eps)   (RMS + q-norm fold)
"""

import contextlib

import ml_dtypes
import numpy as np

import concourse.bass as bass
import concourse.mybir as mybir
import concourse.tile as tile
from concourse import bacc

F32 = mybir.dt.float32
BF16 = mybir.dt.bfloat16
AF = mybir.ActivationFunctionType
ALU = mybir.AluOpType

B, L, D, H, HD, CONV = 4, 2048, 1024, 8, 128, 4
EPS = 1e-6
C = 128
NCH = L // C
NLT = 4
LT = 512
HL = 4
KS = D // 128
SIG = ("k", "q", "v")
LN2H = 0.34657359  # ln(2)/2


def build_nc():
    nc = bacc.Bacc("TRN2", target_bir_lowering=False, debug=False)

    xt_d = nc.dram_tensor("xt", [KS, 128, L], BF16, kind="ExternalInput").ap()
    wq_d = nc.dram_tensor("wq", [KS, 128, 512], BF16, kind="ExternalInput").ap()
    wk_d = nc.dram_tensor("wk", [KS, 128, 512], BF16, kind="ExternalInput").ap()
    wv_d = nc.dram_tensor("wv", [KS, 128, 512], BF16, kind="ExternalInput").ap()
    wb_d = nc.dram_tensor("wb", [KS, 128, 4], BF16, kind="ExternalInput").ap()
    wo_d = nc.dram_tensor("wo", [4, 128, 1024], BF16, kind="ExternalInput").ap()
    cd_d = nc.dram_tensor("cd", [12, CONV, 128, 128], BF16, kind="ExternalInput").ap()
    mkrep_d = nc.dram_tensor("mkrep", [128, 1024], BF16, kind="ExternalInput").ap()
    oh_d = nc.dram_tensor("oh", [4, 128, 4], BF16, kind="ExternalInput").ap()
    id16_d = nc.dram_tensor("id16", [128, 128], BF16, kind="ExternalInput").ap()
    sel_d = nc.dram_tensor("sel", [16, 16, 128], BF16, kind="ExternalInput").ap()
    out_d = nc.dram_tensor("out", [L, D], F32, kind="ExternalOutput").ap()

    with tile.TileContext(nc) as tc, contextlib.ExitStack() as ctx:
        consts = ctx.enter_context(tc.tile_pool(name="consts", bufs=1))
        persist = ctx.enter_context(tc.tile_pool(name="persist", bufs=1))
        xtp = ctx.enter_context(tc.tile_pool(name="xtp", bufs=2))
        projp = ctx.enter_context(tc.tile_pool(name="projp", bufs=1))
        halop = ctx.enter_context(tc.tile_pool(name="halop", bufs=2))
        rawp = ctx.enter_context(tc.tile_pool(name="rawp", bufs=2))
        bcp = ctx.enter_context(tc.tile_pool(name="bcp", bufs=2))
        sqp = ctx.enter_context(tc.tile_pool(name="sqp", bufs=2))
        scalp = ctx.enter_context(tc.tile_pool(name="scalp", bufs=2))
        chkp = ctx.enter_context(tc.tile_pool(name="chkp", bufs=3))
        chainp = ctx.enter_context(tc.tile_pool(name="chainp", bufs=2))
        outp = ctx.enter_context(tc.tile_pool(name="outp", bufs=2))
        # PSUM banks (8): big 2 + pst 2 + gq 1 + ch_sm 1 + ch_a 2
        ps_big = ctx.enter_context(tc.tile_pool(name="ps_big", bufs=2, space="PSUM"))
        ps_t = ctx.enter_context(tc.tile_pool(name="ps_t", bufs=2, space="PSUM"))
        ps_gq = ctx.enter_context(tc.tile_pool(name="ps_gq", bufs=1, space="PSUM"))
        ps_ch = ctx.enter_context(tc.tile_pool(name="ps_ch", bufs=1, space="PSUM"))
        ps_cha = ctx.enter_context(tc.tile_pool(name="ps_cha", bufs=2, space="PSUM"))

        # ---- constants ----
        ws = {}
        for name, d in (("q", wq_d), ("k", wk_d), ("v", wv_d)):
            w = consts.tile([128, KS, 512], BF16, name=f"w{name}")
            for i in range(KS):
                nc.sync.dma_start(out=w[:, i, :], in_=d[i])
            ws[name] = w
        wb = consts.tile([128, KS, 4], BF16, name="wb")
        for i in range(KS):
            nc.sync.dma_start(out=wb[:, i, :], in_=wb_d[i])
        wo = consts.tile([128, 4, 1024], BF16, name="wo")
        for i in range(4):
            nc.sync.dma_start(out=wo[:, i, :], in_=wo_d[i])
        cd = consts.tile([128, 12, CONV, 128], BF16, name="cd")
        for n_ in range(12):
            for j_ in range(CONV):
                nc.sync.dma_start(out=cd[:, n_, j_, :], in_=cd_d[n_, j_])
        mkrep = consts.tile([128, 8, 128], BF16, name="mkrep")
        for n_ in range(8):
            nc.sync.dma_start(out=mkrep[:, n_, :],
                              in_=mkrep_d[:, n_ * 128:(n_ + 1) * 128])
        oh = consts.tile([128, 4, 4], BF16, name="oh")
        for n_ in range(4):
            nc.sync.dma_start(out=oh[:, n_, :], in_=oh_d[n_])
        id16 = consts.tile([128, 128], BF16, name="id16")
        nc.sync.dma_start(out=id16, in_=id16_d)
        sel = consts.tile([16, 16, 128], BF16, name="sel")
        for n_ in range(16):
            nc.sync.dma_start(out=sel[:, n_, :], in_=sel_d[:, n_])
        onescol = consts.tile([128, 1], BF16, name="onescol")
        nc.vector.memset(onescol, 1.0)
        ln2b = consts.tile([4, 1], F32, name="ln2b")
        nc.vector.memset(ln2b, LN2H)
        epsb = consts.tile([128, 1], F32, name="epsb")
        nc.vector.memset(epsb, EPS)

        # ---- persistent ----
        # per-token column-major scalars, one [4, L] tile each
        rkb_cm = persist.tile([4, L], BF16, name="rkb_cm")
        bp_cm = persist.tile([4, L], BF16, name="bp_cm")
        rq_cm = persist.tile([4, L], BF16, name="rq_cm")
        rk_cm = persist.tile([4, L], BF16, name="rk_cm")
        s16_init = persist.tile([128, HL, 128], BF16, name="s16i")
        nc.vector.memset(s16_init, 0.0)

        prev = {}  # software-pipeline state (conv halos)

        def emit_proj_sig(lt, st, s):
            """projection + conv (+ norm scalars) for one signal s of block lt."""
            tsl = bass.ds(lt * LT, LT)
            si = SIG.index(s)
            if "xt" not in st:
                xt = xtp.tile([128, KS, LT], BF16, name="xt", tag="xt")
                for i in range(KS):
                    nc.sync.dma_start(out=xt[:, i, :], in_=xt_d[i, :, tsl])
                st["xt"] = xt
            xt = st["xt"]
            pts = []
            for h in range(HL):
                ps = ps_big.tile([128, LT], F32, name="psproj", tag="big")
                for i in range(KS):
                    nc.tensor.matmul(
                        ps, ws[s][:, i, h * 128:(h + 1) * 128], xt[:, i, :],
                        start=(i == 0), stop=(i == KS - 1))
                pt = projp.tile([128, LT + 4], BF16, name="pt", tag=f"pj{s}{h}")
                if lt == 0:
                    nc.vector.memset(pt[:, 0:3], 0.0)
                else:
                    nc.vector.tensor_copy(pt[:, 0:3], prev[("halo", s, h)])
                nc.scalar.copy(pt[:, 3:LT + 3], ps)
                halo = halop.tile([128, 3], BF16, name="halo", tag=f"hl{s}{h}")
                nc.vector.tensor_copy(halo, pt[:, LT:LT + 3])
                prev[("halo", s, h)] = halo
                pts.append(pt)
            psn = None
            raw = st["raw"]
            for h in range(HL):
                n = si * HL + h
                pc = ps_big.tile([128, LT], F32, name="psconv", tag="big")
                for j in range(CONV):
                    nc.tensor.matmul(pc, cd[:, n, j, :], pts[h][:, j:LT + j],
                                     start=(j == 0), stop=(j == CONV - 1))
                if s not in raw:
                    raw[s] = rawp.tile([128, HL, LT], BF16, name="raw",
                                       tag=f"rw{s}")
                r = raw[s][:, h, :]
                if s == "v":
                    nc.scalar.activation(r, pc, AF.Silu)
                else:
                    nc.scalar.copy(r, pc)
                    sq = sqp.tile([128, LT], BF16, name="sq", tag="sqs")
                    nc.vector.tensor_mul(sq, r, r)
                    if psn is None:
                        psn = ps_gq.tile([4, LT], F32, name=f"pn{s}", tag="gq")
                    nc.tensor.matmul(psn, oh[:, h, :], sq,
                                     start=(h == 0), stop=(h == HL - 1))
            if s in ("k", "q"):
                nrm = scalp.tile([4, LT], F32, name="nrm", tag="nrm")
                nc.scalar.activation(nrm, psn, AF.Sqrt)
                rf = scalp.tile([4, LT], F32, name="rf", tag="rf")
                nc.vector.reciprocal_approx_fast(rf, nrm)
                dst = rk_cm if s == "k" else rq_cm
                nc.vector.tensor_copy(dst[:, tsl], rf)

        def emit_proj_tail(lt, st):
            """beta projection, beta', rkb and broadcast tiles for block lt."""
            tsl = bass.ds(lt * LT, LT)
            xt = st["xt"]
            psb = ps_gq.tile([4, LT], F32, name="psbeta", tag="gq")
            for i in range(KS):
                nc.tensor.matmul(psb, wb[:, i, :], xt[:, i, :],
                                 start=(i == 0), stop=(i == KS - 1))
            tnh = scalp.tile([4, LT], BF16, name="tnh", tag="tnh")
            nc.scalar.activation(tnh, psb, AF.Tanh, scale=0.5, bias=ln2b)
            nc.vector.tensor_scalar(bp_cm[:, tsl], tnh, 0.25, 0.25,
                                    op0=ALU.mult, op1=ALU.add)
            nc.vector.tensor_mul(rkb_cm[:, tsl], rk_cm[:, tsl], bp_cm[:, tsl])
            for nm_, src_cm in (("rk", rk_cm), ("rkb", rkb_cm)):
                bc = bcp.tile([128, HL, LT], BF16, name=f"{nm_}bc",
                              tag=f"{nm_}bc")
                for h in range(HL):
                    pb = ps_big.tile([128, LT], F32, name="psbc", tag="big")
                    nc.tensor.matmul(pb, sel[0:4, h, :], src_cm[:, tsl])
                    (nc.vector.tensor_copy if h % 2 == 0 else nc.scalar.copy)(
                        bc[:, h, :], pb)
                st[nm_ + "bc"] = bc

        def emit_chunk_a(cidx, st):
            """phase A for chunk cidx; returns state for phase B."""
            cc = cidx % 4
            csl = bass.ds(cc * C, C)
            raw = st["raw"]

            # per-token rows: bt cols 0-3 rkb, 4-7 b, 8-11 rq
            gsl = bass.ds(cidx * C, C)
            psbt = ps_t.tile([128, 12], BF16, name="psbt", tag="pst")
            nc.tensor.transpose(psbt[:, 0:4], rkb_cm[:, gsl], id16[0:4, 0:4])
            nc.tensor.transpose(psbt[:, 4:8], bp_cm[:, gsl], id16[0:4, 0:4])
            nc.tensor.transpose(psbt[:, 8:12], rq_cm[:, gsl], id16[0:4, 0:4])
            bt = chkp.tile([128, 12], F32, name="bt", tag="bt")
            nc.vector.tensor_copy(bt, psbt)

            # normalized k / k' / rk-scaled silu(v), raw [dim, tok] layout
            kn4 = chkp.tile([128, HL, C], BF16, name="kn4", tag="kn4")
            kb4 = chkp.tile([128, HL, C], BF16, name="kb4", tag="kb4")
            vn4 = chkp.tile([128, HL, C], BF16, name="vn4", tag="vn4")
            rkbc, rkbbc = st["rkbc"], st["rkbbc"]
            nc.vector.tensor_mul(kn4, raw["k"][:, :, csl], rkbc[:, :, csl])
            nc.vector.tensor_mul(kb4, raw["k"][:, :, csl], rkbbc[:, :, csl])
            nc.vector.tensor_mul(vn4, raw["v"][:, :, csl], rkbc[:, :, csl])

            # token-major [tok, dim]: vkt = [rk*silu(v) x4 | kn x4]
            pvk = ps_t.tile([128, 8, C], BF16, name="pvk", tag="pst")
            for h in range(HL):
                nc.tensor.transpose(pvk[:, h, :], vn4[:, h, :], id16)
            for h in range(HL):
                nc.tensor.transpose(pvk[:, 4 + h, :], kn4[:, h, :], id16)
            vkt = chkp.tile([128, 8, C], BF16, name="vkt", tag="vkt")
            nc.vector.tensor_copy(vkt, pvk)

            # gram matrices G'_4, KnQ^T_4 -> masked -> [nt_4 | lo_4]
            ntlo = chkp.tile([128, 8, C], BF16, name="ntlo", tag="ntlo")
            pg = ps_gq.tile([128, HL, C], F32, name="pg", tag="gq")
            for h in range(HL):
                nc.tensor.matmul(pg[:, h, :], kn4[:, h, :], kb4[:, h, :])
            nc.vector.tensor_mul(ntlo[:, 0:4, :], pg, mkrep[:, 0:4, :])
            pq = ps_gq.tile([128, HL, C], F32, name="pq", tag="gq")
            for h in range(HL):
                nc.tensor.matmul(pq[:, h, :], kn4[:, h, :],
                                 raw["q"][:, h, csl])
            nc.vector.tensor_mul(ntlo[:, 4:8, :], pq, mkrep[:, 4:8, :])

            # nm = N via PE transpose of nt
            pnm = ps_t.tile([128, HL, C], BF16, name="pnm", tag="pst")
            for h in range(HL):
                nc.tensor.transpose(pnm[:, h, :], ntlo[:, h, :], id16)
            nm4 = chkp.tile([128, HL, C], BF16, name="nm4", tag="nm4")
            nc.vector.tensor_copy(nm4, pnm)

            # x0 = b * [v' | kn]  (gpsimd, per-head halves)
            x = chainp.tile([128, HL, 256], BF16, name="x", tag="x")
            for h in range(HL):
                nc.vector.tensor_scalar_mul(x[:, h, 0:128], vkt[:, h, :],
                                            bt[:, 4 + h:5 + h])
                nc.vector.tensor_scalar_mul(x[:, h, 128:256], vkt[:, 4 + h, :],
                                            bt[:, 4 + h:5 + h])

            # chain matrices: t1 = (N^2)^T, p1 = N^2, t2 = (N^4)^T
            def mm4_copy(lhs_of, rhs_of, name, evac, pool, tag):
                p = pool.tile([128, HL, C], F32, name=f"ps{name}", tag=tag)
                for h in range(HL):
                    nc.tensor.matmul(p[:, h, :], lhs_of(h), rhs_of(h))
                t = chainp.tile([128, HL, C], BF16, name=name, tag=name)
                evac(t, p)
                return t

            t1 = mm4_copy(lambda h: nm4[:, h, :], lambda h: ntlo[:, h, :],
                          "t1", nc.vector.tensor_copy, ps_ch, "ch_sm")
            p1 = mm4_copy(lambda h: ntlo[:, h, :], lambda h: nm4[:, h, :],
                          "p1", nc.scalar.copy, ps_gq, "gq")
            t2 = mm4_copy(lambda h: p1[:, h, :], lambda h: t1[:, h, :],
                          "t2", nc.vector.tensor_copy, ps_ch, "ch_sm")

            # apply chain: x_next = (I + lev^T) x, accumulated on PE
            # (nt mask is negated host-side so level 1 is I - N)
            for li, lev in enumerate((ntlo, t1, t2)):
                xn = chainp.tile([128, HL, 256], BF16, name="x", tag="x")
                for g in range(2):
                    pa = ps_cha.tile([128, 2, 256], F32, name="psa", tag="ch_a")
                    nc.tensor.matmul(pa, id16, x[:, 2 * g:2 * g + 2, :],
                                     start=True, stop=False)
                    for h in (2 * g, 2 * g + 1):
                        nc.tensor.matmul(pa[:, h - 2 * g, :], lev[:, h, :],
                                         x[:, h, :], start=False,
                                         stop=(h == 2 * g + 1))
                    evac = nc.scalar.copy if g == 0 else nc.vector.tensor_copy
                    evac(xn[:, 2 * g:2 * g + 2, :], pa)
                x = xn

            return dict(cidx=cidx, csl=csl, x=x, ntlo=ntlo, vkt=vkt, raw=raw,
                        bt=bt)

        def emit_chunk_b(st, s16_prev, s16_new):
            cidx, csl, x, ntlo, vkt, raw = (
                st["cidx"], st["csl"], st["x"], st["ntlo"], st["vkt"],
                st["raw"])
            # ukt = (x3_k)^T
            pukt = ps_t.tile([128, HL, C], BF16, name="pukt", tag="pst")
            for h in range(HL):
                nc.tensor.transpose(pukt[:, h, :], x[:, h, 128:256], id16)
            ukt = chainp.tile([128, HL, C], BF16, name="ukt", tag="ukt")
            nc.vector.tensor_copy(ukt, pukt)
            # u = x3_v - Uk' S
            pu = ps_ch.tile([128, HL, C], F32, name="psu", tag="ch_sm")
            for h in range(HL):
                nc.tensor.matmul(pu[:, h, :], ukt[:, h, :], s16_prev[:, h, :])
            u4 = chainp.tile([128, HL, C], BF16, name="u4", tag="u4")
            nc.vector.tensor_sub(u4, x[:, :, 0:128], pu)
            # po^T = u^T lo + S^T q^T
            po = ps_cha.tile([128, HL, C], F32, name="pspo", tag="ch_a")
            for h in range(HL):
                nc.tensor.matmul(po[:, h, :], u4[:, h, :], ntlo[:, 4 + h, :],
                                 start=True, stop=False)
                nc.tensor.matmul(po[:, h, :], s16_prev[:, h, :],
                                 raw["q"][:, h, csl], start=False, stop=True)
            # S_new = id*S + Kn^T u  (bf16 state round-trip)
            pd = ps_ch.tile([128, HL, C], F32, name="psd", tag="ch_sm")
            nc.tensor.matmul(pd, id16, s16_prev, start=True, stop=False)
            for h in range(HL):
                nc.tensor.matmul(pd[:, h, :], vkt[:, 4 + h, :], u4[:, h, :],
                                 start=False, stop=(h == HL - 1))
            nc.scalar.copy(s16_new, pd)
            # RMS + q-norm fold, column-wise
            po_sb = chainp.tile([128, HL, C], BF16, name="po_sb", tag="po_sb")
            nc.scalar.copy(po_sb, po)
            sq4 = chainp.tile([128, HL, C], BF16, name="sq4", tag="sq4")
            nc.vector.tensor_mul(sq4, po_sb, po_sb)
            bt = st["bt"]
            pms4 = ps_t.tile([128, 4], F32, name="pms4", tag="pst")
            for h in range(HL):
                nc.tensor.matmul(pms4[:, h:h + 1], sq4[:, h, :], onescol)
            rq2t = chainp.tile([128, 4], BF16, name="rq2t", tag="rq2t")
            nc.vector.tensor_mul(rq2t, bt[:, 8:12], bt[:, 8:12])
            m2t = chainp.tile([128, 4], F32, name="m2t", tag="m2t")
            nc.vector.tensor_mul(m2t, pms4, rq2t)
            ms2 = chainp.tile([128, 4], F32, name="ms2", tag="ms2")
            nc.scalar.activation(ms2, m2t, AF.Sqrt, scale=1.0 / HD, bias=epsb)
            rot = chainp.tile([128, 4], F32, name="rot", tag="rot")
            nc.vector.reciprocal(rot, ms2)
            ro2t = chainp.tile([128, 4], BF16, name="ro2t", tag="ro2t")
            nc.vector.tensor_mul(ro2t, rot, bt[:, 8:12])
            prt = ps_t.tile([4, 128], BF16, name="prt", tag="pst")
            nc.tensor.transpose(prt, ro2t, id16)
            ror = chainp.tile([4, 128], BF16, name="ror", tag="ror")
            nc.vector.tensor_copy(ror, prt)
            prb = ps_t.tile([128, HL, C], F32, name="psrb", tag="pst")
            for h in range(HL):
                nc.tensor.matmul(prb[:, h, :], sel[0:4, h, :], ror)
            ot = chainp.tile([128, HL, C], BF16, name="ot", tag="ot",
                             bufs=10)
            nc.vector.tensor_mul(ot, po_sb, prb)
            return ot

        def emit_outproj(cidx, ot):
            gsl = bass.ds(cidx * C, C)
            for oc in range(2):
                p = ps_cha.tile([128, 512], F32, name="psop", tag="ch_a")
                for h in range(HL):
                    nc.tensor.matmul(p, ot[:, h, :],
                                     wo[:, h, oc * 512:(oc + 1) * 512],
                                     start=(h == 0), stop=(h == HL - 1))
                stt = outp.tile([128, 512], F32, name="ost", tag="ost")
                nc.scalar.copy(stt, p)
                nc.sync.dma_start(out=out_d[gsl, oc * 512:(oc + 1) * 512],
                                  in_=stt)

        def emit_proj_piece(lt, st, piece):
            if piece < 3:
                emit_proj_sig(lt, st, SIG[piece])
            else:
                emit_proj_tail(lt, st)

        pending = None
        s16_prev = s16_init
        opq = []  # deferred outproj queue: (cidx, ot)
        OPLAG = 5

        def flush_outproj(limit):
            while len(opq) > limit:
                ci, ot_ = opq.pop(0)
                emit_outproj(ci, ot_)

        states = {0: {"raw": {}}}
        for piece in range(4):
            emit_proj_piece(0, states[0], piece)
        for lt in range(NLT):
            if lt + 1 < NLT:
                states[lt + 1] = {"raw": {}}
            for cc in range(4):
                if lt + 1 < NLT:
                    emit_proj_piece(lt + 1, states[lt + 1], cc)
                sta = emit_chunk_a(lt * 4 + cc, states[lt])
                if pending is not None:
                    s16_new = chainp.tile([128, HL, 128], BF16, name="s16",
                                          tag="s16")
                    ot = emit_chunk_b(pending, s16_prev, s16_new)
                    opq.append((pending["cidx"], ot))
                    flush_outproj(OPLAG)
                    s16_prev = s16_new
                pending = sta
        s16_new = chainp.tile([128, HL, 128], BF16, name="s16", tag="s16")
        ot = emit_chunk_b(pending, s16_prev, s16_new)
        opq.append((pending["cidx"], ot))
        flush_outproj(0)

    nc.compile()
    return nc


# ---------------- host side ----------------

def _bf(x):
    return np.ascontiguousarray(np.asarray(x, np.float32)).astype(ml_dtypes.bfloat16)


def host_prep(inputs):
    x = np.asarray(inputs["x"], np.float32)
    rms_vec = np.tile(np.asarray(inputs["rms_w"], np.float32), H)
    wo_eff = np.asarray(inputs["Wo"], np.float32) * rms_vec[None, :]

    m1 = np.triu(np.ones((128, 128), np.float32), 1)
    m0 = np.triu(np.ones((128, 128), np.float32), 0)
    mkrep = np.concatenate([np.tile(-m1, (1, 4)), np.tile(m0, (1, 4))],
                           axis=1).astype(np.float32)
    ident = np.eye(128, dtype=np.float32)
    oneh = np.zeros((4, 128, 4), np.float32)
    for h in range(4):
        oneh[h, :, h] = 1.0
    sel3 = np.zeros((16, 16, 128), np.float32)
    for r in range(16):
        sel3[r, r, :] = 1.0

    for nm in ("bq", "bk", "bv", "bbeta", "bo", "convb_q", "convb_k", "convb_v"):
        assert np.all(np.asarray(inputs[nm]) == 0.0), f"nonzero bias {nm} unsupported"

    in_maps = []
    for c in range(8):
        b, hh = c // 2, c % 2
        rows = slice(hh * 512, (hh + 1) * 512)
        cds = []
        for s in ("k", "q", "v"):
            cw = np.asarray(inputs[f"conv_{s}"], np.float32)[rows]
            for h in range(HL):
                cds.append(np.stack([np.diag(cw[h * 128:(h + 1) * 128, j])
                                     for j in range(CONV)]))
        m = {
            "xt": _bf(x[b].T.reshape(KS, 128, L)),
            "wq": _bf(np.asarray(inputs["Wq"], np.float32)[rows].T.reshape(KS, 128, 512)),
            "wk": _bf(np.asarray(inputs["Wk"], np.float32)[rows].T.reshape(KS, 128, 512)),
            "wv": _bf(np.asarray(inputs["Wv"], np.float32)[rows].T.reshape(KS, 128, 512)),
            "wb": _bf(np.asarray(inputs["Wbeta"], np.float32)[hh * 4:(hh + 1) * 4].T.reshape(KS, 128, 4)),
            "wo": _bf(wo_eff[:, rows].T.reshape(4, 128, 1024)),
            "cd": np.stack(cds).astype(ml_dtypes.bfloat16),
            "mkrep": _bf(mkrep),
            "oh": _bf(oneh),
            "id16": _bf(ident),
            "sel": _bf(sel3),
        }
        in_maps.append(m)
    return in_maps


def host_combine(results, inputs):
    bo = np.asarray(inputs["bo"], np.float32)
    out = np.zeros((B, L, D), np.float32)
    for b in range(B):
        out[b] = results[2 * b]["out"] + results[2 * b + 1]["out"] + bo
    return out


# ---------------- entry point ----------------

_NC_CACHE = []


def kernel(**inputs):
    """Full-input DeltaNet layer distributed over 8 NeuronCores.

    Shards batch (4) x head-group (2) across cores, runs the Bass kernel via
    run_bass_kernel_spmd, and reduces the per-pair partial out-projections on
    the host (the pair all-reduce) before returning [4, 2048, 1024] fp32.
    """
    from concourse.bass_utils import run_bass_kernel_spmd

    if not _NC_CACHE:
        _NC_CACHE.append(build_nc())
    nc = _NC_CACHE[0]
    in_maps = host_prep(inputs)
    br = run_bass_kernel_spmd(nc, in_maps, list(range(8)))
    return host_combine(br.results, inputs)
